# revision 30
# baseline (speedup 1.0000x reference)
"""Group-quantized linear (fake int4 per-group dequant) GEMV on 8 Trainium2 cores.

Reference computation (all fp32):
    qw = round_half_even(clip(W, -8, 7))            # W in [-8, 7) so clip is identity
    out = (qw.reshape(O, 64, 128) * scales[:, :, None]).reshape(O, O) @ x

Sharding: column-parallel — each core owns a 1024-row slice of W/scales,
x replicated, outputs concatenated (per the tensor-parallel hint).

Device pipeline, built around the HBM stream (memory-bound problem):
  DMA   : TWO HW DGE queues (SP + Activation engines) stream the weights
          concurrently (~400-425 GB/s aggregate vs ~310 single-queue;
          the gpsimd SW-DGE as a third queue measured to COLLAPSE the HW
          queues to ~110 GB/s each — don't).  The per-core weight slice
          is shipped pre-packed (host-side layout only) as [chunk, c,
          group, o] so every unit lands with 16 KiB partition-contiguous
          descriptors, and the two queues' units interleave at 16 KiB in
          HBM (measured ~15% faster per queue than disjoint regions).
          Chunks 0-6 move as 2 MiB half-chunk units; the last chunk as
          four staggered 1 MiB units so little compute trails the final
          bytes.  x heads the SP queue, scales the Act queue; deep wf
          buffering (7×2 MiB) keeps DMA triggers ahead of the quantize
          pipeline (5-buf version stalled queue B 9 us at fill time).
  DVE   : quantize via the fp32 magic-number trick (w + 1.5*2^23) -
          1.5*2^23 == round-half-even exactly, cast to bf16 (exact for
          ints in [-8, 7]); one tensor_scalar per unit.
  PE    : per (group g, out-chunk oc) matmul acc[:, oc, g, :2] =
          qw[128c, 128o].T @ x2[128c, 2] where x2 = [x_hi | x_lo] bf16
          Dekker split of x (fp32-accurate), all accumulated in one fp32
          PSUM tile [128, 8, 64, 2] (2 banks).
  DVE   : epilogue out[o] = sum_{g,j} acc[o, oc, g, j] * scales[o, oc, g]
          with hi/lo-duplicated scales, flat 1-free-dim APs (the fused
          tensor_tensor_reduce doesn't compile on this walrus): stage A
          (groups < 56) emitted after the tail quantizes so DVE never
          blocks on PE mid-tail; stage B + combine after the last matmul.
  PE/DVE: transpose [128, 8] result for a contiguous output DMA

Measured (core-0 NTFF profile): 103.0-103.4 us in the device's fast mode;
run-to-run bimodality up to ~118 us (per-queue rate 228 vs 193 B/ns) with
the identical NEFF — shared-HBM noise, not kernel-dependent.
"""

import numpy as np

IN_DIM = 8192
OUT_DIM = 8192
NUM_GROUPS = 64
GROUP_SIZE = 128  # IN_DIM // NUM_GROUPS
N_CORES = 8
PER_OUT = OUT_DIM // N_CORES  # 1024
P = 128
OC_N = PER_OUT // P  # 8

MAGIC = np.float32(12582912.0)  # 1.5 * 2**23: (w + MAGIC) - MAGIC == rint(w)

GPC = 8  # groups per chunk; chunk = the 4 MiB A/B-interleaved layout block
N_CHUNKS = NUM_GROUPS // GPC  # 8
EP_SPLIT = 56  # epilogue stage-A covers groups [0, 56); stage B the last chunk

_cache = {}


def _units():
    """(group_start, n_groups, queue) in stream order; queues: 0=SP HW DGE,
    1=Act HW DGE, 2=gpsimd SW DGE (a third, otherwise-idle DMA path).
    Chunks 0-6 move as 2 MiB half-chunk units (16 KiB partition-contiguous
    descriptors — measured fastest; halves of one chunk interleave at
    16 KiB in HBM, measured ~15% faster per queue than disjoint regions).
    The SW queue takes 8 MiB spread across the middle so the two HW queues
    carry 12 MiB each.  The last chunk moves as four staggered 1 MiB units
    so only ~2 groups of compute trail the final bytes."""
    qmap = {}  # ch -> (qL, qH); SW DGE measured to collapse HW-queue rates
    u = []
    for ch in range(N_CHUNKS - 1):
        ql, qh = qmap.get(ch, (0, 1))
        u.append((ch * GPC, 4, ql))
        u.append((ch * GPC + 4, 4, qh))
    gs = (N_CHUNKS - 1) * GPC
    u += [(gs, 2, 0), (gs + 4, 2, 1), (gs + 2, 2, 0), (gs + 6, 2, 1)]
    return u


def _split_multi_waits(nc):
    """walrus in this container accepts only ONE sync-wait per instruction;
    Tile's tail drain carries one per producer proc. Hoist extras onto
    same-engine NoOps placed immediately before — identical semantics for an
    in-order sequencer."""
    import concourse.mybir as mybir

    uid = 0
    for f in nc.m.functions:
        for blk in f.blocks:
            insts = blk.instructions
            if not any(
                i.sync_info is not None
                and i.sync_info.on_wait
                and len(i.sync_info.on_wait) > 1
                for i in insts
            ):
                continue
            new_insts = []
            for inst in insts:
                si = inst.sync_info
                if si is not None and si.on_wait and len(si.on_wait) > 1:
                    waits = list(si.on_wait)
                    for w in waits[:-1]:
                        uid += 1
                        new_insts.append(
                            mybir.InstNoOp(
                                name=f"I-waitsplit-{uid}",
                                engine=inst.engine,
                                ins=[],
                                outs=[],
                                sync_info=mybir.SyncInfo(on_wait=[w], on_update=[]),
                            )
                        )
                    inst.sync_info = mybir.SyncInfo(
                        on_wait=[waits[-1]], on_update=si.on_update
                    )
                new_insts.append(inst)
            blk.instructions = new_insts
    return nc


def build_nc(w_bufs=7, q_bufs=4, split_waits=True):
    import concourse.bass as bass
    import concourse.mybir as mybir
    import concourse.tile as tile
    from concourse.masks import make_identity

    f32 = mybir.dt.float32
    bf16 = mybir.dt.bfloat16
    add = mybir.AluOpType.add

    ng = NUM_GROUPS

    nc = bass.Bass()
    wt_d = nc.dram_tensor("wt", [IN_DIM * PER_OUT], f32, kind="ExternalInput")
    x_d = nc.dram_tensor("x", [IN_DIM], f32, kind="ExternalInput")
    sc_d = nc.dram_tensor("scales", [P, OC_N, ng], f32, kind="ExternalInput")
    out_d = nc.dram_tensor("out", [PER_OUT], f32, kind="ExternalOutput")

    units = _units()

    with tile.TileContext(nc) as tc:
        with (
            tc.tile_pool(name="singles", bufs=1) as singles,
            tc.tile_pool(name="w", bufs=w_bufs) as wpool,
            tc.tile_pool(name="q", bufs=q_bufs) as qpool,
            tc.tile_pool(name="psum", bufs=1, space="PSUM") as psum,
        ):
            # ---- x heads the SP queue, scales head the Act queue (both
            # tiny); weights flow right behind on both.
            x_nat = singles.tile([ng, GROUP_SIZE], f32)
            nc.sync.dma_start(x_nat, x_d.rearrange("(g c) -> g c", c=GROUP_SIZE))
            sc_sb = singles.tile([P, OC_N, ng], f32)
            nc.scalar.dma_start(sc_sb, sc_d[:])

            # ---- weight stream: unit k on queue k%2 (A=SP, B=Act)
            wt_v = wt_d.rearrange(
                "(ch c g o) -> ch c g o", ch=N_CHUNKS, c=P, g=GPC
            )
            utiles = []
            engines = [nc.sync, nc.scalar, nc.gpsimd]
            for k, (gs, g, q) in enumerate(units):
                wf = wpool.tile(
                    [P, g, PER_OUT], f32, tag=f"wf{g}", name=f"wf{k}",
                    bufs=4 if g == 2 else None,
                )
                ch, a = gs // GPC, gs % GPC
                engines[q].dma_start(wf, wt_v[ch][:, a : a + g, :])
                utiles.append(wf)

            # ---- x prep: PE-transpose [ng,128] -> [128,ng], Dekker-split
            # into interleaved bf16 hi/lo [128, ng, 2].
            ident_g = singles.tile([ng, ng], f32)
            make_identity(nc, ident_g)
            ident_p = singles.tile([P, P], f32)
            make_identity(nc, ident_p)

            x_ps = psum.tile([P, ng], f32, tag="paux")
            nc.tensor.transpose(x_ps, x_nat, ident_g)
            xT = singles.tile([P, ng], f32)
            nc.vector.tensor_copy(out=xT, in_=x_ps)
            xhi = singles.tile([P, ng], bf16)
            nc.vector.tensor_copy(out=xhi, in_=xT)
            xhi32 = singles.tile([P, ng], f32)
            nc.vector.tensor_copy(out=xhi32, in_=xhi)
            xlo32 = singles.tile([P, ng], f32)
            nc.vector.tensor_tensor(xlo32, xT, xhi32, mybir.AluOpType.subtract)
            x2 = singles.tile([P, ng, 2], bf16)
            nc.vector.tensor_copy(out=x2[:, :, 0], in_=xhi)
            nc.vector.tensor_copy(out=x2[:, :, 1], in_=xlo32)

            # one fused PSUM accumulator [128, oc, g, hi/lo] (2 banks)
            acc = psum.tile([P, OC_N, ng, 2], f32, tag="pacc")
            accf = acc.rearrange("p oc g j -> p oc (g j)")

            # sc2 = scales duplicated over hi/lo, for flat epilogue APs
            sc2 = singles.tile([P, OC_N, ng, 2], f32)

            # ---- main loop: per-unit quantize + 8 matmuls per group
            for k, (gs, g, q) in enumerate(units):
                qw = qpool.tile(
                    [P, g, PER_OUT], bf16, tag=f"qw{g}", name=f"qw{k}",
                    bufs=4 if g == 2 else None,
                )
                nc.vector.tensor_scalar(
                    out=qw,
                    in0=utiles[k],
                    scalar1=float(MAGIC),
                    scalar2=-float(MAGIC),
                    op0=add,
                    op1=add,
                )
                for gp in range(g):
                    for oc in range(OC_N):
                        nc.tensor.matmul(
                            acc[:, oc, gs + gp, :],
                            lhsT=qw[:, gp, oc * P : (oc + 1) * P],
                            rhs=x2[:, gs + gp, :],
                            start=True,
                            stop=True,
                        )
                if k == 2:
                    # sc2 prep early — scales landed at the head, DVE is idle
                    nc.vector.tensor_copy(out=sc2[:, :, :, 0], in_=sc_sb)
                    nc.vector.tensor_copy(out=sc2[:, :, :, 1], in_=sc_sb)

            # ---- epilogue: out[o] = sum_{g,j} acc * sc2.  Stage A first
            # (groups < EP_SPLIT; all matmuls for those finished long ago),
            # stage B + combine after the final matmul.
            sc2f = sc2.rearrange("p oc g j -> p oc (g j)")
            es = EP_SPLIT * 2

            ysA = singles.tile([P, OC_N, es], f32)
            nc.vector.tensor_tensor(
                ysA, accf[:, :, :es], sc2f[:, :, :es], mybir.AluOpType.mult
            )
            outA = singles.tile([P, OC_N], f32)
            nc.vector.reduce_sum(
                out=outA.unsqueeze(2), in_=ysA, axis=mybir.AxisListType.X
            )

            ysB = singles.tile([P, OC_N, ng * 2 - es], f32)
            nc.vector.tensor_tensor(
                ysB, accf[:, :, es:], sc2f[:, :, es:], mybir.AluOpType.mult
            )
            outB = singles.tile([P, OC_N], f32)
            nc.vector.reduce_sum(
                out=outB.unsqueeze(2), in_=ysB, axis=mybir.AxisListType.X
            )
            out_sb = singles.tile([P, OC_N], f32)
            nc.vector.tensor_tensor(out_sb, outA, outB, add)

            # ---- transpose [128, oc] -> [oc, 128] for a contiguous store
            o_ps = psum.tile([OC_N, P], f32, tag="paux")
            nc.tensor.transpose(o_ps, out_sb, ident_p)
            outT = singles.tile([OC_N, P], f32)
            nc.vector.tensor_copy(out=outT, in_=o_ps)
            nc.sync.dma_start(out_d.rearrange("(oc p) -> oc p", p=P), outT)

    return _split_multi_waits(nc) if split_waits else nc


def make_in_maps(x, weights, scales):
    """Per-core input staging (host-side layout only)."""
    x = np.ascontiguousarray(np.asarray(x, dtype=np.float32))
    weights = np.asarray(weights, dtype=np.float32)
    scales = np.asarray(scales, dtype=np.float32)
    in_maps = []
    for c in range(N_CORES):
        sl = slice(c * PER_OUT, (c + 1) * PER_OUT)
        wtc = weights[sl].T  # [in_dim, per_out]
        # [ch, c, gp, o]: each partition's chunk data contiguous (32 KiB)
        wt = np.ascontiguousarray(
            wtc.reshape(N_CHUNKS, GPC, P, PER_OUT).transpose(0, 2, 1, 3)
        ).ravel()
        scc = np.ascontiguousarray(
            scales[sl].reshape(OC_N, P, NUM_GROUPS).transpose(1, 0, 2)
        )
        in_maps.append({"wt": wt, "x": x, "scales": scc})
    return in_maps


def kernel(x, weights, scales):
    from concourse import bass_utils

    if "nc" not in _cache:
        _cache["nc"] = build_nc()
    nc = _cache["nc"]

    in_maps = make_in_maps(x, weights, scales)
    res = bass_utils.run_bass_kernel_spmd(nc, in_maps, core_ids=list(range(N_CORES)))
    return np.concatenate([res.results[c]["out"] for c in range(N_CORES)]).astype(
        np.float32
    )


# revision 31
# speedup vs baseline: 1.0135x; 1.0135x over previous
"""Group-quantized linear (fake int4 per-group dequant) GEMV on 8 Trainium2 cores.

Reference computation (all fp32):
    qw = round_half_even(clip(W, -8, 7))            # W in [-8, 7) so clip is identity
    out = (qw.reshape(O, 64, 128) * scales[:, :, None]).reshape(O, O) @ x

Sharding: column-parallel — each core owns a 1024-row slice of W/scales,
x replicated, outputs concatenated (per the tensor-parallel hint).

Device pipeline, built around the HBM stream (memory-bound problem):
  DMA   : TWO HW DGE queues (SP + Activation engines) stream the weights
          concurrently (~400-425 GB/s aggregate vs ~310 single-queue;
          the gpsimd SW-DGE as a third queue measured to COLLAPSE the HW
          queues to ~110 GB/s each — don't).  The per-core weight slice
          is shipped pre-packed (host-side layout only) as [chunk, c,
          group, o] so every unit lands with 16 KiB partition-contiguous
          descriptors, and the two queues' units interleave at 16 KiB in
          HBM (measured ~15% faster per queue than disjoint regions).
          Chunks 0-6 move as 2 MiB half-chunk units; the last chunk as
          four staggered 1 MiB units so little compute trails the final
          bytes.  x heads the SP queue, scales the Act queue; deep wf
          buffering (7×2 MiB) keeps DMA triggers ahead of the quantize
          pipeline (5-buf version stalled queue B 9 us at fill time).
  DVE   : quantize via the fp32 magic-number trick (w + 1.5*2^23) -
          1.5*2^23 == round-half-even exactly, cast to bf16 (exact for
          ints in [-8, 7]); one tensor_scalar per unit.
  PE    : per (group g, out-chunk oc) matmul acc[:, oc, g, :2] =
          qw[128c, 128o].T @ x2[128c, 2] where x2 = [x_hi | x_lo] bf16
          Dekker split of x (fp32-accurate), all accumulated in one fp32
          PSUM tile [128, 8, 64, 2] (2 banks).
  DVE   : epilogue out[o] = sum_{g,j} acc[o, oc, g, j] * scales[o, oc, g]
          with hi/lo-duplicated scales, flat 1-free-dim APs (the fused
          tensor_tensor_reduce doesn't compile on this walrus): stage A
          (groups < 56) emitted after the tail quantizes so DVE never
          blocks on PE mid-tail; stage B + combine after the last matmul.
  PE/DVE: transpose [128, 8] result for a contiguous output DMA

Measured (core-0 NTFF profile): 103.0-103.4 us in the device's fast mode;
run-to-run bimodality up to ~118 us (per-queue rate 228 vs 193 B/ns) with
the identical NEFF — shared-HBM noise, not kernel-dependent.
"""

import numpy as np

IN_DIM = 8192
OUT_DIM = 8192
NUM_GROUPS = 64
GROUP_SIZE = 128  # IN_DIM // NUM_GROUPS
N_CORES = 8
PER_OUT = OUT_DIM // N_CORES  # 1024
P = 128
OC_N = PER_OUT // P  # 8

MAGIC = np.float32(12582912.0)  # 1.5 * 2**23: (w + MAGIC) - MAGIC == rint(w)

GPC = 8  # groups per chunk; chunk = the 4 MiB A/B-interleaved layout block
N_CHUNKS = NUM_GROUPS // GPC  # 8
EP_SPLIT = 56  # epilogue stage-A covers groups [0, 56); stage B the last chunk

_cache = {}


def _units():
    """(group_start, n_groups, queue) in stream order; queues: 0=SP HW DGE,
    1=Act HW DGE, 2=gpsimd SW DGE (a third, otherwise-idle DMA path).
    Chunks 0-6 move as 2 MiB half-chunk units (16 KiB partition-contiguous
    descriptors — measured fastest; halves of one chunk interleave at
    16 KiB in HBM, measured ~15% faster per queue than disjoint regions).
    The SW queue takes 8 MiB spread across the middle so the two HW queues
    carry 12 MiB each.  The last chunk moves as four staggered 1 MiB units
    so only ~2 groups of compute trail the final bytes."""
    qmap = {}  # ch -> (qL, qH); SW DGE measured to collapse HW-queue rates
    u = []
    for ch in range(N_CHUNKS - 1):
        ql, qh = qmap.get(ch, (0, 1))
        u.append((ch * GPC, 4, ql))
        u.append((ch * GPC + 4, 4, qh))
    gs = (N_CHUNKS - 1) * GPC
    u += [(gs, 2, 0), (gs + 4, 2, 1), (gs + 2, 2, 0), (gs + 6, 2, 1)]
    return u


def _split_multi_waits(nc):
    """walrus in this container accepts only ONE sync-wait per instruction;
    Tile's tail drain carries one per producer proc. Hoist extras onto
    same-engine NoOps placed immediately before — identical semantics for an
    in-order sequencer."""
    import concourse.mybir as mybir

    uid = 0
    for f in nc.m.functions:
        for blk in f.blocks:
            insts = blk.instructions
            if not any(
                i.sync_info is not None
                and i.sync_info.on_wait
                and len(i.sync_info.on_wait) > 1
                for i in insts
            ):
                continue
            new_insts = []
            for inst in insts:
                si = inst.sync_info
                if si is not None and si.on_wait and len(si.on_wait) > 1:
                    waits = list(si.on_wait)
                    for w in waits[:-1]:
                        uid += 1
                        new_insts.append(
                            mybir.InstNoOp(
                                name=f"I-waitsplit-{uid}",
                                engine=inst.engine,
                                ins=[],
                                outs=[],
                                sync_info=mybir.SyncInfo(on_wait=[w], on_update=[]),
                            )
                        )
                    inst.sync_info = mybir.SyncInfo(
                        on_wait=[waits[-1]], on_update=si.on_update
                    )
                new_insts.append(inst)
            blk.instructions = new_insts
    return nc


def build_nc(w_bufs=7, q_bufs=4, split_waits=True):
    import concourse.bass as bass
    import concourse.mybir as mybir
    import concourse.tile as tile
    from concourse.masks import make_identity

    f32 = mybir.dt.float32
    bf16 = mybir.dt.bfloat16
    add = mybir.AluOpType.add

    ng = NUM_GROUPS

    nc = bass.Bass()
    wt_d = nc.dram_tensor("wt", [IN_DIM * PER_OUT], f32, kind="ExternalInput")
    x_d = nc.dram_tensor("x", [IN_DIM], f32, kind="ExternalInput")
    sc_d = nc.dram_tensor("scales", [P, OC_N, ng], f32, kind="ExternalInput")
    out_d = nc.dram_tensor("out", [PER_OUT], f32, kind="ExternalOutput")

    units = _units()

    with tile.TileContext(nc) as tc:
        with (
            tc.tile_pool(name="singles", bufs=1) as singles,
            tc.tile_pool(name="w", bufs=w_bufs) as wpool,
            tc.tile_pool(name="q", bufs=q_bufs) as qpool,
            tc.tile_pool(name="psum", bufs=1, space="PSUM") as psum,
        ):
            # ---- x heads the SP queue, scales head the Act queue (both
            # tiny); weights flow right behind on both.
            x_nat = singles.tile([ng, GROUP_SIZE], f32)
            nc.sync.dma_start(x_nat, x_d.rearrange("(g c) -> g c", c=GROUP_SIZE))
            sc_sb = singles.tile([P, OC_N, ng], f32)
            nc.scalar.dma_start(sc_sb, sc_d[:])

            # ---- weight stream: unit k on queue k%2 (A=SP, B=Act)
            wt_v = wt_d.rearrange(
                "(ch c g o) -> ch c g o", ch=N_CHUNKS, c=P, g=GPC
            )
            utiles = []
            engines = [nc.sync, nc.scalar, nc.gpsimd]
            for k, (gs, g, q) in enumerate(units):
                wf = wpool.tile(
                    [P, g, PER_OUT], f32, tag=f"wf{g}", name=f"wf{k}",
                    bufs=4 if g == 2 else None,
                )
                ch, a = gs // GPC, gs % GPC
                engines[q].dma_start(wf, wt_v[ch][:, a : a + g, :])
                utiles.append(wf)

            # ---- x prep: PE-transpose [ng,128] -> [128,ng], Dekker-split
            # into interleaved bf16 hi/lo [128, ng, 2].
            ident_g = singles.tile([ng, ng], f32)
            make_identity(nc, ident_g)
            ident_p = singles.tile([P, P], f32)
            make_identity(nc, ident_p)

            x_ps = psum.tile([P, ng], f32, tag="paux")
            nc.tensor.transpose(x_ps, x_nat, ident_g)
            xT = singles.tile([P, ng], f32)
            nc.vector.tensor_copy(out=xT, in_=x_ps)
            xhi = singles.tile([P, ng], bf16)
            nc.vector.tensor_copy(out=xhi, in_=xT)
            xhi32 = singles.tile([P, ng], f32)
            nc.vector.tensor_copy(out=xhi32, in_=xhi)
            xlo32 = singles.tile([P, ng], f32)
            nc.vector.tensor_tensor(xlo32, xT, xhi32, mybir.AluOpType.subtract)
            x2 = singles.tile([P, ng, 2], bf16)
            nc.vector.tensor_copy(out=x2[:, :, 0], in_=xhi)
            nc.vector.tensor_copy(out=x2[:, :, 1], in_=xlo32)

            # three PSUM accumulators split at group 48 and 56 so each
            # epilogue stage's read depends only on the matmuls it covers
            # (a single tile made stage A wait on ALL 512 matmuls —
            # tile-granular dependency tracking)
            S1, S2 = 48, EP_SPLIT  # acc0: g<48, acc1: 48..55, acc2: 56..63
            acc0 = psum.tile([P, OC_N, S1, 2], f32, tag="pacc0")
            acc1 = psum.tile([P, OC_N, S2 - S1, 2], f32, tag="pacc1")
            acc2 = psum.tile([P, OC_N, ng - S2, 2], f32, tag="pacc2")

            def acc_slot(gg):
                if gg < S1:
                    return acc0[:, :, gg, :]
                if gg < S2:
                    return acc1[:, :, gg - S1, :]
                return acc2[:, :, gg - S2, :]

            # sc2 = scales duplicated over hi/lo, for flat epilogue APs
            sc2 = singles.tile([P, OC_N, ng, 2], f32)
            sc2f = sc2.rearrange("p oc g j -> p oc (g j)")
            outA1 = singles.tile([P, OC_N], f32)

            # ---- main loop: per-unit quantize + 8 matmuls per group
            for k, (gs, g, q) in enumerate(units):
                qw = qpool.tile(
                    [P, g, PER_OUT], bf16, tag=f"qw{g}", name=f"qw{k}",
                    bufs=4 if g == 2 else None,
                )
                nc.vector.tensor_scalar(
                    out=qw,
                    in0=utiles[k],
                    scalar1=float(MAGIC),
                    scalar2=-float(MAGIC),
                    op0=add,
                    op1=add,
                )
                for gp in range(g):
                    gg = gs + gp
                    dst = acc_slot(gg)
                    for oc in range(OC_N):
                        nc.tensor.matmul(
                            dst[:, oc, :],
                            lhsT=qw[:, gp, oc * P : (oc + 1) * P],
                            rhs=x2[:, gg, :],
                            start=True,
                            stop=True,
                        )
                if k == 2:
                    # sc2 prep early — scales landed at the head, DVE is idle
                    nc.vector.tensor_copy(out=sc2[:, :, :, 0], in_=sc_sb)
                    nc.vector.tensor_copy(out=sc2[:, :, :, 1], in_=sc_sb)
                if k == len(units) - 3:
                    # epilogue stage A1 (groups < 48): slots into the DVE
                    # idle window between the 2nd and 3rd tail quantizes;
                    # its matmuls (chunks 0-5) finished long ago
                    ysA1 = singles.tile([P, OC_N, S1 * 2], f32)
                    nc.vector.tensor_tensor(
                        ysA1,
                        acc0.rearrange("p oc g j -> p oc (g j)"),
                        sc2f[:, :, : S1 * 2],
                        mybir.AluOpType.mult,
                    )
                    nc.vector.reduce_sum(
                        out=outA1.unsqueeze(2), in_=ysA1, axis=mybir.AxisListType.X
                    )

            # ---- epilogue stages A2 (chunk 6) and B (chunk 7) + combine:
            # only ~1.5 us of DVE after the final matmul
            ysA2 = singles.tile([P, OC_N, (S2 - S1) * 2], f32)
            nc.vector.tensor_tensor(
                ysA2,
                acc1.rearrange("p oc g j -> p oc (g j)"),
                sc2f[:, :, S1 * 2 : S2 * 2],
                mybir.AluOpType.mult,
            )
            outA2 = singles.tile([P, OC_N], f32)
            nc.vector.reduce_sum(
                out=outA2.unsqueeze(2), in_=ysA2, axis=mybir.AxisListType.X
            )

            ysB = singles.tile([P, OC_N, (ng - S2) * 2], f32)
            nc.vector.tensor_tensor(
                ysB,
                acc2.rearrange("p oc g j -> p oc (g j)"),
                sc2f[:, :, S2 * 2 :],
                mybir.AluOpType.mult,
            )
            outB = singles.tile([P, OC_N], f32)
            nc.vector.reduce_sum(
                out=outB.unsqueeze(2), in_=ysB, axis=mybir.AxisListType.X
            )
            outA = singles.tile([P, OC_N], f32)
            nc.vector.tensor_tensor(outA, outA1, outA2, add)
            out_sb = singles.tile([P, OC_N], f32)
            nc.vector.tensor_tensor(out_sb, outA, outB, add)

            # ---- transpose [128, oc] -> [oc, 128] for a contiguous store
            o_ps = psum.tile([OC_N, P], f32, tag="paux")
            nc.tensor.transpose(o_ps, out_sb, ident_p)
            outT = singles.tile([OC_N, P], f32)
            nc.vector.tensor_copy(out=outT, in_=o_ps)
            nc.sync.dma_start(out_d.rearrange("(oc p) -> oc p", p=P), outT)

    return _split_multi_waits(nc) if split_waits else nc


def make_in_maps(x, weights, scales):
    """Per-core input staging (host-side layout only)."""
    x = np.ascontiguousarray(np.asarray(x, dtype=np.float32))
    weights = np.asarray(weights, dtype=np.float32)
    scales = np.asarray(scales, dtype=np.float32)
    in_maps = []
    for c in range(N_CORES):
        sl = slice(c * PER_OUT, (c + 1) * PER_OUT)
        wtc = weights[sl].T  # [in_dim, per_out]
        # [ch, c, gp, o]: each partition's chunk data contiguous (32 KiB)
        wt = np.ascontiguousarray(
            wtc.reshape(N_CHUNKS, GPC, P, PER_OUT).transpose(0, 2, 1, 3)
        ).ravel()
        scc = np.ascontiguousarray(
            scales[sl].reshape(OC_N, P, NUM_GROUPS).transpose(1, 0, 2)
        )
        in_maps.append({"wt": wt, "x": x, "scales": scc})
    return in_maps


def kernel(x, weights, scales):
    from concourse import bass_utils

    if "nc" not in _cache:
        _cache["nc"] = build_nc()
    nc = _cache["nc"]

    in_maps = make_in_maps(x, weights, scales)
    res = bass_utils.run_bass_kernel_spmd(nc, in_maps, core_ids=list(range(N_CORES)))
    return np.concatenate([res.results[c]["out"] for c in range(N_CORES)]).astype(
        np.float32
    )


# revision 32
# speedup vs baseline: 1.1171x; 1.1022x over previous
"""Group-quantized linear (fake int4 per-group dequant) GEMV on 8 Trainium2 cores.

Reference computation (all fp32):
    qw = round_half_even(clip(W, -8, 7))            # W in [-8, 7) so clip is identity
    out = (qw.reshape(O, 64, 128) * scales[:, :, None]).reshape(O, O) @ x

Sharding: column-parallel — each core owns a 1024-row slice of W/scales,
x replicated, outputs concatenated (per the tensor-parallel hint).

Device pipeline, built around the HBM stream (memory-bound problem):
  DMA   : TWO HW DGE queues (SP + Activation engines) stream the weights
          concurrently (~400-425 GB/s aggregate vs ~310 single-queue;
          the gpsimd SW-DGE as a third queue measured to COLLAPSE the HW
          queues to ~110 GB/s each — don't).  The per-core weight slice
          is shipped pre-packed (host-side layout only) as [chunk, c,
          group, o] so every unit lands with 16 KiB partition-contiguous
          descriptors, and the two queues' units interleave at 16 KiB in
          HBM (measured ~15% faster per queue than disjoint regions).
          Chunks 0-6 move as 2 MiB half-chunk units; the last chunk as
          four staggered 1 MiB units so little compute trails the final
          bytes.  x heads the SP queue, scales the Act queue; deep wf
          buffering (7×2 MiB) keeps DMA triggers ahead of the quantize
          pipeline (5-buf version stalled queue B 9 us at fill time).
  DVE   : quantize via the fp32 magic-number trick (w + 1.5*2^23) -
          1.5*2^23 == round-half-even exactly, cast to bf16 (exact for
          ints in [-8, 7]); one tensor_scalar per unit.
  PE    : per (group g, out-chunk oc) matmul acc[:, oc, g, :2] =
          qw[128c, 128o].T @ x2[128c, 2] where x2 = [x_hi | x_lo] bf16
          Dekker split of x (fp32-accurate), accumulated in THREE fp32
          PSUM tiles split at groups 48 and 56 so each epilogue stage's
          read depends only on the matmuls it covers (one fused tile made
          stage A wait on all 512 matmuls — tile-granular deps).
  DVE   : epilogue out[o] = sum_{g,j} acc[o, oc, g, j] * scales[o, oc, g]
          with hi/lo-duplicated scales, flat 1-free-dim APs (the fused
          tensor_tensor_reduce doesn't compile on this walrus): stage A1
          (groups < 48) slots into the DVE idle window between tail
          quantizes; stages A2 + B + combine are ~1.5 us after the last
          matmul.
  PE/DVE: transpose [128, 8] result for a contiguous output DMA

Measured (core-0 NTFF profile): 101.2 us in the device's fast mode
(per-queue DMA ~228 B/ns); run-to-run drift up to ~115 us (~195-205 B/ns)
with the identical NEFF — shared-HBM noise, not kernel-dependent.
"""

import numpy as np

IN_DIM = 8192
OUT_DIM = 8192
NUM_GROUPS = 64
GROUP_SIZE = 128  # IN_DIM // NUM_GROUPS
N_CORES = 8
PER_OUT = OUT_DIM // N_CORES  # 1024
P = 128
OC_N = PER_OUT // P  # 8

MAGIC = np.float32(12582912.0)  # 1.5 * 2**23: (w + MAGIC) - MAGIC == rint(w)

GPC = 8  # groups per chunk; chunk = the 4 MiB A/B-interleaved layout block
N_CHUNKS = NUM_GROUPS // GPC  # 8
EP_SPLIT = 56  # epilogue stage-A covers groups [0, 56); stage B the last chunk

_cache = {}


def _units():
    """(group_start, n_groups, queue) in stream order; queues: 0=SP HW DGE,
    1=Act HW DGE, 2=gpsimd SW DGE (a third, otherwise-idle DMA path).
    Chunks 0-6 move as 2 MiB half-chunk units (16 KiB partition-contiguous
    descriptors — measured fastest; halves of one chunk interleave at
    16 KiB in HBM, measured ~15% faster per queue than disjoint regions).
    The SW queue takes 8 MiB spread across the middle so the two HW queues
    carry 12 MiB each.  The last chunk moves as four staggered 1 MiB units
    so only ~2 groups of compute trail the final bytes."""
    qmap = {}  # ch -> (qL, qH); SW DGE measured to collapse HW-queue rates
    u = []
    for ch in range(N_CHUNKS - 1):
        ql, qh = qmap.get(ch, (0, 1))
        u.append((ch * GPC, 4, ql))
        u.append((ch * GPC + 4, 4, qh))
    gs = (N_CHUNKS - 1) * GPC
    u += [(gs, 2, 0), (gs + 4, 2, 1), (gs + 2, 2, 0), (gs + 6, 2, 1)]
    return u


def _split_multi_waits(nc):
    """walrus in this container accepts only ONE sync-wait per instruction;
    Tile's tail drain carries one per producer proc. Hoist extras onto
    same-engine NoOps placed immediately before — identical semantics for an
    in-order sequencer."""
    import concourse.mybir as mybir

    uid = 0
    for f in nc.m.functions:
        for blk in f.blocks:
            insts = blk.instructions
            if not any(
                i.sync_info is not None
                and i.sync_info.on_wait
                and len(i.sync_info.on_wait) > 1
                for i in insts
            ):
                continue
            new_insts = []
            for inst in insts:
                si = inst.sync_info
                if si is not None and si.on_wait and len(si.on_wait) > 1:
                    waits = list(si.on_wait)
                    for w in waits[:-1]:
                        uid += 1
                        new_insts.append(
                            mybir.InstNoOp(
                                name=f"I-waitsplit-{uid}",
                                engine=inst.engine,
                                ins=[],
                                outs=[],
                                sync_info=mybir.SyncInfo(on_wait=[w], on_update=[]),
                            )
                        )
                    inst.sync_info = mybir.SyncInfo(
                        on_wait=[waits[-1]], on_update=si.on_update
                    )
                new_insts.append(inst)
            blk.instructions = new_insts
    return nc


def build_nc(w_bufs=7, q_bufs=4, split_waits=True):
    import concourse.bass as bass
    import concourse.mybir as mybir
    import concourse.tile as tile
    from concourse.masks import make_identity

    f32 = mybir.dt.float32
    bf16 = mybir.dt.bfloat16
    add = mybir.AluOpType.add

    ng = NUM_GROUPS

    nc = bass.Bass()
    wt_d = nc.dram_tensor("wt", [IN_DIM * PER_OUT], f32, kind="ExternalInput")
    x_d = nc.dram_tensor("x", [IN_DIM], f32, kind="ExternalInput")
    sc_d = nc.dram_tensor("scales", [P, OC_N, ng], f32, kind="ExternalInput")
    out_d = nc.dram_tensor("out", [PER_OUT], f32, kind="ExternalOutput")

    units = _units()

    with tile.TileContext(nc) as tc:
        with (
            tc.tile_pool(name="singles", bufs=1) as singles,
            tc.tile_pool(name="w", bufs=w_bufs) as wpool,
            tc.tile_pool(name="q", bufs=q_bufs) as qpool,
            tc.tile_pool(name="psum", bufs=1, space="PSUM") as psum,
        ):
            # ---- x heads the SP queue, scales head the Act queue (both
            # tiny); weights flow right behind on both.
            x_nat = singles.tile([ng, GROUP_SIZE], f32)
            nc.sync.dma_start(x_nat, x_d.rearrange("(g c) -> g c", c=GROUP_SIZE))
            sc_sb = singles.tile([P, OC_N, ng], f32)
            nc.scalar.dma_start(sc_sb, sc_d[:])

            # ---- weight stream: unit k on queue k%2 (A=SP, B=Act)
            wt_v = wt_d.rearrange(
                "(ch c g o) -> ch c g o", ch=N_CHUNKS, c=P, g=GPC
            )
            utiles = []
            engines = [nc.sync, nc.scalar, nc.gpsimd]
            for k, (gs, g, q) in enumerate(units):
                wf = wpool.tile(
                    [P, g, PER_OUT], f32, tag=f"wf{g}", name=f"wf{k}",
                    bufs=4 if g == 2 else None,
                )
                ch, a = gs // GPC, gs % GPC
                engines[q].dma_start(wf, wt_v[ch][:, a : a + g, :])
                utiles.append(wf)

            # ---- x prep: PE-transpose [ng,128] -> [128,ng], Dekker-split
            # into interleaved bf16 hi/lo [128, ng, 2].
            ident_g = singles.tile([ng, ng], f32)
            make_identity(nc, ident_g)
            ident_p = singles.tile([P, P], f32)
            make_identity(nc, ident_p)

            x_ps = psum.tile([P, ng], f32, tag="paux")
            nc.tensor.transpose(x_ps, x_nat, ident_g)
            xT = singles.tile([P, ng], f32)
            nc.vector.tensor_copy(out=xT, in_=x_ps)
            xhi = singles.tile([P, ng], bf16)
            nc.vector.tensor_copy(out=xhi, in_=xT)
            xhi32 = singles.tile([P, ng], f32)
            nc.vector.tensor_copy(out=xhi32, in_=xhi)
            xlo32 = singles.tile([P, ng], f32)
            nc.vector.tensor_tensor(xlo32, xT, xhi32, mybir.AluOpType.subtract)
            x2 = singles.tile([P, ng, 2], bf16)
            nc.vector.tensor_copy(out=x2[:, :, 0], in_=xhi)
            nc.vector.tensor_copy(out=x2[:, :, 1], in_=xlo32)

            # three PSUM accumulators split at group 48 and 56 so each
            # epilogue stage's read depends only on the matmuls it covers
            # (a single tile made stage A wait on ALL 512 matmuls —
            # tile-granular dependency tracking)
            S1, S2 = 48, EP_SPLIT  # acc0: g<48, acc1: 48..55, acc2: 56..63
            acc0 = psum.tile([P, OC_N, S1, 2], f32, tag="pacc0")
            acc1 = psum.tile([P, OC_N, S2 - S1, 2], f32, tag="pacc1")
            acc2 = psum.tile([P, OC_N, ng - S2, 2], f32, tag="pacc2")

            def acc_slot(gg):
                if gg < S1:
                    return acc0[:, :, gg, :]
                if gg < S2:
                    return acc1[:, :, gg - S1, :]
                return acc2[:, :, gg - S2, :]

            # sc2 = scales duplicated over hi/lo, for flat epilogue APs
            sc2 = singles.tile([P, OC_N, ng, 2], f32)
            sc2f = sc2.rearrange("p oc g j -> p oc (g j)")
            outA1 = singles.tile([P, OC_N], f32)

            # ---- main loop: per-unit quantize + 8 matmuls per group
            for k, (gs, g, q) in enumerate(units):
                qw = qpool.tile(
                    [P, g, PER_OUT], bf16, tag=f"qw{g}", name=f"qw{k}",
                    bufs=4 if g == 2 else None,
                )
                nc.vector.tensor_scalar(
                    out=qw,
                    in0=utiles[k],
                    scalar1=float(MAGIC),
                    scalar2=-float(MAGIC),
                    op0=add,
                    op1=add,
                )
                for gp in range(g):
                    gg = gs + gp
                    dst = acc_slot(gg)
                    for oc in range(OC_N):
                        nc.tensor.matmul(
                            dst[:, oc, :],
                            lhsT=qw[:, gp, oc * P : (oc + 1) * P],
                            rhs=x2[:, gg, :],
                            start=True,
                            stop=True,
                        )
                if k == 2:
                    # sc2 prep early — scales landed at the head, DVE is idle
                    nc.vector.tensor_copy(out=sc2[:, :, :, 0], in_=sc_sb)
                    nc.vector.tensor_copy(out=sc2[:, :, :, 1], in_=sc_sb)
                if k == len(units) - 3:
                    # epilogue stage A1 (groups < 48): slots into the DVE
                    # idle window between the 2nd and 3rd tail quantizes;
                    # its matmuls (chunks 0-5) finished long ago
                    ysA1 = singles.tile([P, OC_N, S1 * 2], f32)
                    nc.vector.tensor_tensor(
                        ysA1,
                        acc0.rearrange("p oc g j -> p oc (g j)"),
                        sc2f[:, :, : S1 * 2],
                        mybir.AluOpType.mult,
                    )
                    nc.vector.reduce_sum(
                        out=outA1.unsqueeze(2), in_=ysA1, axis=mybir.AxisListType.X
                    )

            # ---- epilogue stages A2 (chunk 6) and B (chunk 7) + combine:
            # only ~1.5 us of DVE after the final matmul
            ysA2 = singles.tile([P, OC_N, (S2 - S1) * 2], f32)
            nc.vector.tensor_tensor(
                ysA2,
                acc1.rearrange("p oc g j -> p oc (g j)"),
                sc2f[:, :, S1 * 2 : S2 * 2],
                mybir.AluOpType.mult,
            )
            outA2 = singles.tile([P, OC_N], f32)
            nc.vector.reduce_sum(
                out=outA2.unsqueeze(2), in_=ysA2, axis=mybir.AxisListType.X
            )

            ysB = singles.tile([P, OC_N, (ng - S2) * 2], f32)
            nc.vector.tensor_tensor(
                ysB,
                acc2.rearrange("p oc g j -> p oc (g j)"),
                sc2f[:, :, S2 * 2 :],
                mybir.AluOpType.mult,
            )
            outB = singles.tile([P, OC_N], f32)
            nc.vector.reduce_sum(
                out=outB.unsqueeze(2), in_=ysB, axis=mybir.AxisListType.X
            )
            outA = singles.tile([P, OC_N], f32)
            nc.vector.tensor_tensor(outA, outA1, outA2, add)
            out_sb = singles.tile([P, OC_N], f32)
            nc.vector.tensor_tensor(out_sb, outA, outB, add)

            # ---- transpose [128, oc] -> [oc, 128] for a contiguous store
            o_ps = psum.tile([OC_N, P], f32, tag="paux")
            nc.tensor.transpose(o_ps, out_sb, ident_p)
            outT = singles.tile([OC_N, P], f32)
            nc.vector.tensor_copy(out=outT, in_=o_ps)
            nc.sync.dma_start(out_d.rearrange("(oc p) -> oc p", p=P), outT)

    return _split_multi_waits(nc) if split_waits else nc


def make_in_maps(x, weights, scales):
    """Per-core input staging (host-side layout only)."""
    x = np.ascontiguousarray(np.asarray(x, dtype=np.float32))
    weights = np.asarray(weights, dtype=np.float32)
    scales = np.asarray(scales, dtype=np.float32)
    in_maps = []
    for c in range(N_CORES):
        sl = slice(c * PER_OUT, (c + 1) * PER_OUT)
        wtc = weights[sl].T  # [in_dim, per_out]
        # [ch, c, gp, o]: each partition's chunk data contiguous (32 KiB)
        wt = np.ascontiguousarray(
            wtc.reshape(N_CHUNKS, GPC, P, PER_OUT).transpose(0, 2, 1, 3)
        ).ravel()
        scc = np.ascontiguousarray(
            scales[sl].reshape(OC_N, P, NUM_GROUPS).transpose(1, 0, 2)
        )
        in_maps.append({"wt": wt, "x": x, "scales": scc})
    return in_maps


def kernel(x, weights, scales):
    from concourse import bass_utils

    if "nc" not in _cache:
        _cache["nc"] = build_nc()
    nc = _cache["nc"]

    in_maps = make_in_maps(x, weights, scales)
    res = bass_utils.run_bass_kernel_spmd(nc, in_maps, core_ids=list(range(N_CORES)))
    return np.concatenate([res.results[c]["out"] for c in range(N_CORES)]).astype(
        np.float32
    )
